# revision 1
# baseline (speedup 1.0000x reference)
"""MinimalRNNCell on 8 Trainium2 NeuronCores.

h_t = x_t @ W + h_{t-1} @ R, h_0 = 0, for x: [B=32, T=1024, D=512],
W: [D, U=512], R: [U, U]. Returns all h_t -> [B, T, U] float32.

Strategy (data-parallel over batch, chunked linear scan over time):
  - Shard batch over 8 cores (BLOC=4 rows each). All matmul work runs in
    the transposed layout h^T[U, r] with r = (chunk j, batch b); the host
    pre-permutes x into xr[d, c, r], t = j*C + c, so every DMA is
    contiguous.
  - C=32 chunks advance simultaneously as the N=128 moving columns of
    each matmul. All matmul operands are bf16 (full PE rate at any N;
    f32r would drop to 1/4 rate below N=256, and mixed 16/32-bit inputs
    are rejected by the compiler). PSUM accumulation stays f32.
  - Phase A: C sequential steps; step c computes, for each 128-row
    output block u, one PSUM accumulation group
      hl_c[u] = sum_d W[d,u]^T x_c[d] + sum_v R[v,u]^T hl_{c-1}[v].
    PSUM allows only one open accumulation group per 2KB bank, so each
    u-group gets its own bank; a group's drain (PSUM -> bf16 state tile,
    alternating DVE/ACT) is emitted right after its stop matmul, and the
    next step's xW matmuls cover the drain latency, keeping the PE at
    ~100% in steady state.
  - R is contractive (||R^32|| ~ 1e-5), so inter-chunk carries reduce to
    e_j = hl_{j-1, C-1} (no Kogge-Stone scan), and corrections
    G_{j,c} = e_j @ R^{c+1} are truncated at c < K (~12, chosen on the
    host from ||R^k|| norms; truncation error ~4e-3 max-rel). Powers
    R^{c+1} are host-precomputed bf16 stationaries, so all corrections
    are independent fat matmuls reading the carry source straight from
    the state tile; the one-chunk shift happens in the output add.
  - Outputs are written bf16 (within precision budget; the state is
    already bf16) and upcast to f32 on the host; offsets c >= K DMA
    straight from the state tiles with no extra copy.
"""

import os

import numpy as np

import concourse.bass as bass
import concourse.mybir as mybir
import concourse.tile as tile
from concourse import bass_utils

B, T, D, U = 32, 1024, 512, 512
NCORES = 8
BLOC = B // NCORES  # 4 batch rows per core
C = int(os.environ.get("RNN_C", "32"))  # chunk length = phase A steps
L = T // C  # 32 chunks
RCOLS = BLOC * L  # 128 moving columns
NCH = U // 128  # 4 partition blocks of the 512-dim
XG = 8  # chunks per x-load DMA
KTOL = 3.5e-2  # drop corrections e @ R^{c+1} once ||R^{c+1}||_2 <= KTOL
FG = 4  # phase-A steps fused into one xW matmul / PSUM bank
MAX_SYNC_WAITS = 1

F32R = mybir.dt.float32r
BF16 = mybir.dt.bfloat16
F32 = mybir.dt.float32


def _split_sync_waits(nc, max_waits=MAX_SYNC_WAITS):
    """Walrus rejects instructions carrying more than a couple of sync
    waits. Hoist excess waits onto single-wait NoOps placed immediately
    before the offending instruction."""
    for fn in nc.m.functions:
        for bb in fn.blocks:
            insts = bb.instructions
            out, changed = [], False
            for inst in insts:
                si = inst.sync_info
                waits = list(si.on_wait) if si is not None else []
                if len(waits) > max_waits:
                    for k, w in enumerate(waits[:-max_waits]):
                        out.append(
                            mybir.InstNoOp(
                                name=f"I-wsplit-{inst.name}-{k}",
                                engine=inst.engine,
                                ins=[],
                                outs=[],
                                sync_info=mybir.SyncInfo(on_wait=[w], on_update=[]),
                            )
                        )
                    inst.sync_info = mybir.SyncInfo(
                        on_wait=waits[-max_waits:], on_update=list(si.on_update)
                    )
                    changed = True
                out.append(inst)
            if changed:
                insts[:] = out


def _build_nc(kcorr, reps=1):
    nc = bass.Bass("TRN2", target_bir_lowering=False, debug=False)
    xr_d = nc.dram_tensor("xr", [D, C, RCOLS], BF16, kind="ExternalInput").ap()
    w_d = nc.dram_tensor("w", [D, U], BF16, kind="ExternalInput").ap()
    r_d = nc.dram_tensor("r", [U, U], BF16, kind="ExternalInput").ap()
    pw_d = nc.dram_tensor("pw", [U, kcorr, U], BF16, kind="ExternalInput").ap()
    hr_d = nc.dram_tensor("hr", [U, C, RCOLS], BF16, kind="ExternalOutput").ap()

    with tile.TileContext(nc) as tc:
      for _rep in range(reps):
        with (
            tc.tile_pool(name=f"wts{_rep}", bufs=1) as wpool,
            tc.tile_pool(name=f"x{_rep}", bufs=1) as xpool,
            tc.tile_pool(name=f"hl{_rep}", bufs=1) as hlpool,
            tc.tile_pool(name=f"e{_rep}", bufs=1) as epool,
            tc.tile_pool(name=f"out{_rep}", bufs=2) as outpool,
            tc.tile_pool(name=f"psA{_rep}", bufs=5, space="PSUM") as psa,
            tc.tile_pool(name=f"psC{_rep}", bufs=3, space="PSUM") as psc,
        ):
            # --- resident weights; band DMAs spread across engines so the
            # first matmul isn't gated on one sequencer ---
            def load_bands(src, name, dt, engines, split_first=False):
                views = [[None] * NCH for _ in range(NCH)]
                for a in range(NCH):
                    t = wpool.tile([128, U], dt, tag=f"{name}{a}", name=f"{name}{a}")
                    if split_first and a == 0:
                        # col-piece DMAs so the first matmul's stationary
                        # block lands quickly
                        for bi in range(NCH):
                            engines[bi % len(engines)].dma_start(
                                out=t[:, 128 * bi : 128 * (bi + 1)],
                                in_=src[128 * a : 128 * (a + 1),
                                        128 * bi : 128 * (bi + 1)],
                            )
                    else:
                        engines[a % len(engines)].dma_start(
                            out=t[:], in_=src[128 * a : 128 * (a + 1), :]
                        )
                    for bi in range(NCH):
                        views[a][bi] = t[:, 128 * bi : 128 * (bi + 1)]
                return views

            w_t = load_bands(w_d, "w", BF16, [nc.scalar, nc.gpsimd], split_first=True)
            r_t = load_bands(r_d, "r", BF16, [nc.scalar, nc.gpsimd])

            # x in XG-chunk groups
            xt = [[None] * (C // XG) for _ in range(NCH)]
            for g in range(C // XG):
                for d in range(NCH):
                    t = xpool.tile(
                        [128, XG, RCOLS], BF16, tag=f"x{d}_{g}", name=f"x{d}_{g}"
                    )
                    nc.sync.dma_start(
                        out=t[:],
                        in_=xr_d[128 * d : 128 * (d + 1), g * XG : (g + 1) * XG, :],
                    )
                    xt[d][g] = t

            def xf(c0, d):
                # FG consecutive chunk-steps as one [128, FG*RCOLS] moving AP
                return xt[d][c0 // XG][:, c0 % XG : c0 % XG + FG, :]

            # correction powers (bf16, stationary): pw[v, c, u]
            pw_t = []
            for v in range(NCH):
                t = wpool.tile([128, kcorr, U], BF16, tag=f"pw{v}", name=f"pw{v}")
                nc.gpsimd.dma_start(
                    out=t[:], in_=pw_d[128 * v : 128 * (v + 1), :, :]
                )
                pw_t.append(t)

            # state: one mega-tile [128, u-block, c, r] so each step drains
            # with two wide copies (DVE half + ACT half in parallel)
            hla = hlpool.tile([128, NCH, C, RCOLS], BF16, tag="hla", name="hla")

            # --- phase A: intra-chunk local scan, FG steps per super-step ---
            # one whole-bank [128, FG, RCOLS] PSUM tile per u-group (PSUM
            # allows only one open accumulation group per bank): the fused
            # xW matmul (N = FG*RCOLS) opens the group for all FG slices,
            # the per-slice recurrence matmuls accumulate into their slice,
            # and each slice drains right after its own u-group finishes
            for c0 in range(0, C, FG):
                pss = [
                    psa.tile([128, FG, RCOLS], F32, tag="ps", name="psA")
                    for _ in range(NCH)
                ]
                for u in range(NCH):
                    for d in range(NCH):
                        nc.tensor.matmul(
                            pss[u][:], w_t[d][u], xf(c0, d),
                            start=(d == 0), stop=False,
                            skip_group_check=True,
                        )
                for k in range(FG):
                    c = c0 + k
                    for u in range(NCH):
                        if c > 0:
                            for v in range(NCH):
                                nc.tensor.matmul(
                                    pss[u][:, k, :], r_t[v][u],
                                    hla[:, v, c - 1, :],
                                    start=False,
                                    stop=(v == NCH - 1 and k == FG - 1),
                                    skip_group_check=True,
                                )
                        if u % 2 == 0:
                            nc.vector.tensor_copy(
                                out=hla[:, u, c, :], in_=pss[u][:, k, :]
                            )
                        else:
                            nc.scalar.copy(
                                out=hla[:, u, c, :], in_=pss[u][:, k, :]
                            )
                    # stream uncorrected outputs (c >= kcorr) as each
                    # XG-aligned group's last step drains
                    if c >= kcorr and (c % XG == XG - 1 or c == C - 1):
                        cs = max(kcorr, c // XG * XG)
                        for u in range(NCH):
                            nc.sync.dma_start(
                                out=hr_d[128 * u : 128 * (u + 1), cs : c + 1, :],
                                in_=hla[:, u, cs : c + 1, :],
                            )

            # --- corrections: h_{j,c} = hl_{j,c} + hl_{j-1,C-1} @ R^{c+1},
            # c < kcorr. The carry source is read straight from the state
            # tile; the chunk shift happens in the output add (psum column i
            # corrects output column i+BLOC; chunk 0 has no correction).
            NS = RCOLS - BLOC  # correction matmul moving width
            CG = 4  # correction steps per PSUM tile / output DMA
            for c0 in range(0, kcorr, CG):
                n = min(CG, kcorr - c0)
                for u in range(NCH):
                    ps = psc.tile([128, CG, NS], F32, tag="pso", name="psoC")
                    for ci in range(n):
                        for v in range(NCH):
                            nc.tensor.matmul(
                                ps[:, ci, :],
                                pw_t[v][:, c0 + ci, 128 * u : 128 * (u + 1)],
                                hla[:, v, C - 1, 0:NS],
                                start=(v == 0), stop=(v == NCH - 1),
                            )
                    ot = outpool.tile([128, CG, RCOLS], BF16, tag=f"o{u}", name=f"o{u}")
                    nc.scalar.copy(
                        out=ot[:, 0:n, 0:BLOC], in_=hla[:, u, c0 : c0 + n, 0:BLOC]
                    )
                    nc.vector.tensor_add(
                        out=ot[:, 0:n, BLOC:RCOLS], in0=ps[:, 0:n, :],
                        in1=hla[:, u, c0 : c0 + n, BLOC:RCOLS],
                    )
                    nc.sync.dma_start(
                        out=hr_d[128 * u : 128 * (u + 1), c0 : c0 + n, :],
                        in_=ot[:, 0:n, :],
                    )

    _split_sync_waits(nc)
    return nc


_CACHE = {}


def _get_nc(kcorr, reps=1):
    key = (kcorr, reps)
    if key not in _CACHE:
        _CACHE[key] = _build_nc(kcorr, reps)
    return _CACHE[key]


def _tf32_round(a):
    b = np.ascontiguousarray(a, np.float32).view(np.uint32)
    r = ((b >> np.uint32(13)) & np.uint32(1)) + np.uint32(0x0FFF)
    b = (b + r) & np.uint32(0xFFFFE000)
    return b.view(np.float32)


def _bf16(a):
    import ml_dtypes

    return np.ascontiguousarray(np.asarray(a, np.float32).astype(ml_dtypes.bfloat16))


def prepare_inputs(x, kernel, recurrent_kernel):
    """Host-side shard + permute. Returns (in_maps, kcorr)."""
    x = np.asarray(x)
    w = np.asarray(kernel)
    r = np.asarray(recurrent_kernel)
    # correction depth + power ladder R^{c+1}, c = 0..kcorr-1 (fp64)
    r64 = r.astype(np.float64)
    pows, m, kcorr = [], r64.copy(), 0
    while kcorr < C:
        if np.linalg.norm(m, 2) <= KTOL and kcorr >= 4:
            break
        pows.append(m)
        m = m @ r64
        kcorr += 1
    # pw[v, c, u] = R^{c+1}[v, u]
    pw = _bf16(np.stack(pows, axis=1))
    wq = _bf16(w)
    rq = _bf16(r)
    in_maps = []
    for k in range(NCORES):
        xc = x[BLOC * k : BLOC * (k + 1)]  # [BLOC, T, D]
        # xr[d, c, j*BLOC + b] = xc[b, j*C + c, d]
        xr = _bf16(
            xc.reshape(BLOC, L, C, D).transpose(3, 2, 1, 0).reshape(D, C, RCOLS)
        )
        in_maps.append({"xr": xr, "w": wq, "r": rq, "pw": pw})
    return in_maps, kcorr


def assemble_output(results):
    out = np.empty((B, T, U), np.float32)
    for k in range(NCORES):
        hr = np.asarray(results[k]["hr"], dtype=np.float32)  # [U, C, RCOLS]
        # out[b, j*C + c, u] = hr[u, c, j*BLOC + b]
        out[BLOC * k : BLOC * (k + 1)] = (
            hr.reshape(U, C, L, BLOC).transpose(3, 2, 1, 0).reshape(BLOC, T, U)
        )
    return out


_RUNNERS = {}


def _get_runner(nc):
    """Build (once) a sharded jitted executable for `nc` on 8 cores."""
    if nc in _RUNNERS:
        return _RUNNERS[nc]
    import jax
    from jax.sharding import Mesh, PartitionSpec
    from jax.experimental.shard_map import shard_map
    from concourse import bass2jax

    bass2jax.install_neuronx_cc_hook()
    partition_name = nc.partition_id_tensor.name if nc.partition_id_tensor else None
    in_names, out_names, out_avals = [], [], []
    for alloc in nc.m.functions[0].allocations:
        if not isinstance(alloc, mybir.MemoryLocationSet):
            continue
        name = alloc.memorylocations[0].name
        if alloc.kind == "ExternalInput":
            if name != partition_name:
                in_names.append(name)
        elif alloc.kind == "ExternalOutput":
            out_names.append(name)
            out_avals.append(
                jax.core.ShapedArray(
                    tuple(alloc.tensor_shape), mybir.dt.np(alloc.dtype)
                )
            )
    n_params = len(in_names)
    in_names_all = list(in_names) + out_names
    if partition_name is not None:
        in_names_all.append(partition_name)

    def _body(*args):
        operands = list(args)
        if partition_name is not None:
            operands.append(bass2jax.partition_id_tensor())
        return tuple(
            bass2jax._bass_exec_p.bind(
                *operands,
                out_avals=tuple(out_avals),
                in_names=tuple(in_names_all),
                out_names=tuple(out_names),
                lowering_input_output_aliases=(),
                sim_require_finite=True,
                sim_require_nnan=True,
                nc=nc,
            )
        )

    devices = jax.devices()[:NCORES]
    mesh = Mesh(np.asarray(devices), ("core",))
    nouts = len(out_names)
    sharded = jax.jit(
        shard_map(
            _body,
            mesh=mesh,
            in_specs=(PartitionSpec("core"),) * (n_params + nouts),
            out_specs=(PartitionSpec("core"),) * nouts,
            check_rep=False,
        ),
        keep_unused=True,
    )

    def run(in_maps):
        concat_in = [
            np.concatenate([np.asarray(in_maps[c][nm]) for c in range(NCORES)], axis=0)
            for nm in in_names
        ]
        concat_zero = [
            np.zeros((NCORES * a.shape[0], *a.shape[1:]), a.dtype) for a in out_avals
        ]
        outs = sharded(*concat_in, *concat_zero)
        return [
            {
                nm: np.asarray(outs[i]).reshape(NCORES, *out_avals[i].shape)[c]
                for i, nm in enumerate(out_names)
            }
            for c in range(NCORES)
        ]

    run.sharded = sharded
    run.in_names = list(in_names)
    run.out_shapes = [(tuple(a.shape), a.dtype) for a in out_avals]
    _RUNNERS[nc] = run
    return run


def kernel(x, kernel, recurrent_kernel):
    in_maps, kcorr = prepare_inputs(x, kernel, recurrent_kernel)
    nc = _get_nc(kcorr)
    results = _get_runner(nc)(in_maps)
    return assemble_output(results)



# revision 2
# speedup vs baseline: 15.8409x; 15.8409x over previous
"""MinimalRNNCell on 8 Trainium2 NeuronCores.

h_t = x_t @ W + h_{t-1} @ R, h_0 = 0, for x: [B=32, T=1024, D=512],
W: [D, U=512], R: [U, U]. Returns all h_t -> [B, T, U] float32.

Strategy (data-parallel over batch, chunked linear scan over time):
  - Shard batch over 8 cores (BLOC=4 rows each). All matmul work runs in
    the transposed layout h^T[U, r] with r = (chunk j, batch b); the host
    pre-permutes x into xr[d, c, r], t = j*C + c, so every DMA is
    contiguous.
  - C=32 chunks advance simultaneously as the N=128 moving columns of
    each matmul. All matmul operands are bf16 (full PE rate at any N;
    f32r would drop to 1/4 rate below N=256, and mixed 16/32-bit inputs
    are rejected by the compiler). PSUM accumulation stays f32.
  - Phase A: C sequential steps; step c computes, for each 128-row
    output block u, one PSUM accumulation group
      hl_c[u] = sum_d W[d,u]^T x_c[d] + sum_v R[v,u]^T hl_{c-1}[v].
    PSUM allows only one open accumulation group per 2KB bank, so each
    u-group gets its own bank; a group's drain (PSUM -> bf16 state tile,
    alternating DVE/ACT) is emitted right after its stop matmul, and the
    next step's xW matmuls cover the drain latency, keeping the PE at
    ~100% in steady state.
  - R is contractive (||R^32|| ~ 1e-5), so inter-chunk carries reduce to
    e_j = hl_{j-1, C-1} (no Kogge-Stone scan), and corrections
    G_{j,c} = e_j @ R^{c+1} are truncated at c < K (~12, chosen on the
    host from ||R^k|| norms; truncation error ~4e-3 max-rel). Powers
    R^{c+1} are host-precomputed bf16 stationaries, so all corrections
    are independent fat matmuls reading the carry source straight from
    the state tile; the one-chunk shift happens in the output add.
  - Outputs are written bf16 (within precision budget; the state is
    already bf16) and upcast to f32 on the host; offsets c >= K DMA
    straight from the state tiles with no extra copy.
"""

import os

import numpy as np

import concourse.bass as bass
import concourse.mybir as mybir
import concourse.tile as tile
from concourse import bass_utils

B, T, D, U = 32, 1024, 512, 512
NCORES = 8
BLOC = B // NCORES  # 4 batch rows per core
C = int(os.environ.get("RNN_C", "64"))  # chunk length = phase A steps
L = T // C  # chunks
RCOLS = BLOC * L  # moving columns
NCH = U // 128  # 4 partition blocks of the 512-dim
XG = int(os.environ.get("RNN_XG", "8"))  # chunks per x-load DMA
KTOL = float(os.environ.get("RNN_KTOL", "6.5e-2"))  # drop e @ R^{c+1} once ||R^{c+1}||_2 <= KTOL
FG = int(os.environ.get("RNN_FG", "8"))  # phase-A steps fused into one xW matmul / PSUM bank
MAX_SYNC_WAITS = 1

F32R = mybir.dt.float32r
BF16 = mybir.dt.bfloat16
F32 = mybir.dt.float32


def _split_sync_waits(nc, max_waits=MAX_SYNC_WAITS):
    """Walrus rejects instructions carrying more than a couple of sync
    waits. Hoist excess waits onto single-wait NoOps placed immediately
    before the offending instruction."""
    for fn in nc.m.functions:
        for bb in fn.blocks:
            insts = bb.instructions
            out, changed = [], False
            for inst in insts:
                si = inst.sync_info
                waits = list(si.on_wait) if si is not None else []
                if len(waits) > max_waits:
                    for k, w in enumerate(waits[:-max_waits]):
                        out.append(
                            mybir.InstNoOp(
                                name=f"I-wsplit-{inst.name}-{k}",
                                engine=inst.engine,
                                ins=[],
                                outs=[],
                                sync_info=mybir.SyncInfo(on_wait=[w], on_update=[]),
                            )
                        )
                    inst.sync_info = mybir.SyncInfo(
                        on_wait=waits[-max_waits:], on_update=list(si.on_update)
                    )
                    changed = True
                out.append(inst)
            if changed:
                insts[:] = out


def _build_nc(kcorr, reps=1):
    nc = bass.Bass("TRN2", target_bir_lowering=False, debug=False)
    xr_d = nc.dram_tensor("xr", [D, C, RCOLS], BF16, kind="ExternalInput").ap()
    w_d = nc.dram_tensor("w", [D, U], BF16, kind="ExternalInput").ap()
    r_d = nc.dram_tensor("r", [U, U], BF16, kind="ExternalInput").ap()
    pw_d = nc.dram_tensor("pw", [U, kcorr, U], BF16, kind="ExternalInput").ap()
    hr_d = nc.dram_tensor("hr", [U, C, RCOLS], BF16, kind="ExternalOutput").ap()

    with tile.TileContext(nc) as tc:
      for _rep in range(reps):
        with (
            tc.tile_pool(name=f"wts{_rep}", bufs=1) as wpool,
            tc.tile_pool(name=f"x{_rep}", bufs=1) as xpool,
            tc.tile_pool(name=f"hl{_rep}", bufs=1) as hlpool,
            tc.tile_pool(name=f"e{_rep}", bufs=1) as epool,
            tc.tile_pool(name=f"out{_rep}", bufs=2) as outpool,
            tc.tile_pool(name=f"psA{_rep}", bufs=5, space="PSUM") as psa,
            tc.tile_pool(name=f"psC{_rep}", bufs=3, space="PSUM") as psc,
        ):
            # --- resident weights; band DMAs spread across engines so the
            # first matmul isn't gated on one sequencer ---
            def load_bands(src, name, dt, engines, split_first=False):
                views = [[None] * NCH for _ in range(NCH)]
                for a in range(NCH):
                    t = wpool.tile([128, U], dt, tag=f"{name}{a}", name=f"{name}{a}")
                    if split_first and a == 0:
                        # col-piece DMAs so the first matmul's stationary
                        # block lands quickly
                        for bi in range(NCH):
                            engines[bi % len(engines)].dma_start(
                                out=t[:, 128 * bi : 128 * (bi + 1)],
                                in_=src[128 * a : 128 * (a + 1),
                                        128 * bi : 128 * (bi + 1)],
                            )
                    else:
                        engines[a % len(engines)].dma_start(
                            out=t[:], in_=src[128 * a : 128 * (a + 1), :]
                        )
                    for bi in range(NCH):
                        views[a][bi] = t[:, 128 * bi : 128 * (bi + 1)]
                return views

            w_t = load_bands(w_d, "w", BF16, [nc.scalar, nc.gpsimd], split_first=True)
            r_t = load_bands(r_d, "r", BF16, [nc.scalar, nc.gpsimd])

            # x in XG-chunk groups
            xt = [[None] * (C // XG) for _ in range(NCH)]
            for g in range(C // XG):
                for d in range(NCH):
                    t = xpool.tile(
                        [128, XG, RCOLS], BF16, tag=f"x{d}_{g}", name=f"x{d}_{g}"
                    )
                    nc.sync.dma_start(
                        out=t[:],
                        in_=xr_d[128 * d : 128 * (d + 1), g * XG : (g + 1) * XG, :],
                    )
                    xt[d][g] = t

            def xf(c0, d):
                # FG consecutive chunk-steps as one [128, FG*RCOLS] moving AP
                return xt[d][c0 // XG][:, c0 % XG : c0 % XG + FG, :]

            # correction powers (bf16, stationary): pw[v, c, u]
            pw_t = []
            for v in range(NCH):
                t = wpool.tile([128, kcorr, U], BF16, tag=f"pw{v}", name=f"pw{v}")
                nc.gpsimd.dma_start(
                    out=t[:], in_=pw_d[128 * v : 128 * (v + 1), :, :]
                )
                pw_t.append(t)

            # state: one mega-tile [128, u-block, c, r] so each step drains
            # with two wide copies (DVE half + ACT half in parallel)
            hla = hlpool.tile([128, NCH, C, RCOLS], BF16, tag="hla", name="hla")

            # --- phase A: intra-chunk local scan, FG steps per super-step ---
            # one whole-bank [128, FG, RCOLS] PSUM tile per u-group (PSUM
            # allows only one open accumulation group per bank): the fused
            # xW matmul (N = FG*RCOLS) opens the group for all FG slices,
            # the per-slice recurrence matmuls accumulate into their slice,
            # and each slice drains right after its own u-group finishes
            for c0 in range(0, C, FG):
                pss = [
                    psa.tile([128, FG, RCOLS], F32, tag="ps", name="psA")
                    for _ in range(NCH)
                ]
                for u in range(NCH):
                    for d in range(NCH):
                        nc.tensor.matmul(
                            pss[u][:], w_t[d][u], xf(c0, d),
                            start=(d == 0), stop=False,
                            skip_group_check=True,
                        )
                for k in range(FG):
                    c = c0 + k
                    for u in range(NCH):
                        if c > 0:
                            for v in range(NCH):
                                nc.tensor.matmul(
                                    pss[u][:, k, :], r_t[v][u],
                                    hla[:, v, c - 1, :],
                                    start=False,
                                    stop=(v == NCH - 1 and k == FG - 1),
                                    skip_group_check=True,
                                )
                        if u % 2 == 0:
                            nc.vector.tensor_copy(
                                out=hla[:, u, c, :], in_=pss[u][:, k, :]
                            )
                        else:
                            nc.scalar.copy(
                                out=hla[:, u, c, :], in_=pss[u][:, k, :]
                            )
                    # stream uncorrected outputs (c >= kcorr) as each
                    # XG-aligned group's last step drains
                    if c >= kcorr and (c % XG == XG - 1 or c == C - 1):
                        cs = max(kcorr, c // XG * XG)
                        for u in range(NCH):
                            nc.sync.dma_start(
                                out=hr_d[128 * u : 128 * (u + 1), cs : c + 1, :],
                                in_=hla[:, u, cs : c + 1, :],
                            )

            # --- corrections: h_{j,c} = hl_{j,c} + hl_{j-1,C-1} @ R^{c+1},
            # c < kcorr. The carry source is read straight from the state
            # tile; the chunk shift happens in the output add (psum column i
            # corrects output column i+BLOC; chunk 0 has no correction).
            NS = RCOLS - BLOC  # correction matmul moving width
            CG = 4  # correction steps per PSUM tile / output DMA
            for c0 in range(0, kcorr, CG):
                n = min(CG, kcorr - c0)
                for u in range(NCH):
                    ps = psc.tile([128, CG, NS], F32, tag="pso", name="psoC")
                    for ci in range(n):
                        for v in range(NCH):
                            nc.tensor.matmul(
                                ps[:, ci, :],
                                pw_t[v][:, c0 + ci, 128 * u : 128 * (u + 1)],
                                hla[:, v, C - 1, 0:NS],
                                start=(v == 0), stop=(v == NCH - 1),
                            )
                    ot = outpool.tile([128, CG, RCOLS], BF16, tag=f"o{u}", name=f"o{u}")
                    nc.scalar.copy(
                        out=ot[:, 0:n, 0:BLOC], in_=hla[:, u, c0 : c0 + n, 0:BLOC]
                    )
                    nc.vector.tensor_add(
                        out=ot[:, 0:n, BLOC:RCOLS], in0=ps[:, 0:n, :],
                        in1=hla[:, u, c0 : c0 + n, BLOC:RCOLS],
                    )
                    nc.sync.dma_start(
                        out=hr_d[128 * u : 128 * (u + 1), c0 : c0 + n, :],
                        in_=ot[:, 0:n, :],
                    )

    _split_sync_waits(nc)
    return nc


_CACHE = {}


def _get_nc(kcorr, reps=1):
    key = (kcorr, reps)
    if key not in _CACHE:
        _CACHE[key] = _build_nc(kcorr, reps)
    return _CACHE[key]


def _tf32_round(a):
    b = np.ascontiguousarray(a, np.float32).view(np.uint32)
    r = ((b >> np.uint32(13)) & np.uint32(1)) + np.uint32(0x0FFF)
    b = (b + r) & np.uint32(0xFFFFE000)
    return b.view(np.float32)


def _bf16(a):
    import ml_dtypes

    return np.ascontiguousarray(np.asarray(a, np.float32).astype(ml_dtypes.bfloat16))


def prepare_inputs(x, kernel, recurrent_kernel):
    """Host-side shard + permute. Returns (in_maps, kcorr)."""
    x = np.asarray(x)
    w = np.asarray(kernel)
    r = np.asarray(recurrent_kernel)
    # correction depth + power ladder R^{c+1}, c = 0..kcorr-1 (fp64)
    r64 = r.astype(np.float64)
    pows, m, kcorr = [], r64.copy(), 0
    while kcorr < C:
        if np.linalg.norm(m, 2) <= KTOL and kcorr >= 4:
            break
        pows.append(m)
        m = m @ r64
        kcorr += 1
    # pw[v, c, u] = R^{c+1}[v, u]
    pw = _bf16(np.stack(pows, axis=1))
    wq = _bf16(w)
    rq = _bf16(r)
    in_maps = []
    for k in range(NCORES):
        xc = x[BLOC * k : BLOC * (k + 1)]  # [BLOC, T, D]
        # xr[d, c, j*BLOC + b] = xc[b, j*C + c, d]
        xr = _bf16(
            xc.reshape(BLOC, L, C, D).transpose(3, 2, 1, 0).reshape(D, C, RCOLS)
        )
        in_maps.append({"xr": xr, "w": wq, "r": rq, "pw": pw})
    return in_maps, kcorr


def assemble_output(results):
    out = np.empty((B, T, U), np.float32)
    for k in range(NCORES):
        hr = np.asarray(results[k]["hr"], dtype=np.float32)  # [U, C, RCOLS]
        # out[b, j*C + c, u] = hr[u, c, j*BLOC + b]
        out[BLOC * k : BLOC * (k + 1)] = (
            hr.reshape(U, C, L, BLOC).transpose(3, 2, 1, 0).reshape(BLOC, T, U)
        )
    return out


_RUNNERS = {}


def _get_runner(nc):
    """Build (once) a sharded jitted executable for `nc` on 8 cores."""
    if nc in _RUNNERS:
        return _RUNNERS[nc]
    import jax
    from jax.sharding import Mesh, PartitionSpec
    from jax.experimental.shard_map import shard_map
    from concourse import bass2jax

    bass2jax.install_neuronx_cc_hook()
    partition_name = nc.partition_id_tensor.name if nc.partition_id_tensor else None
    in_names, out_names, out_avals = [], [], []
    for alloc in nc.m.functions[0].allocations:
        if not isinstance(alloc, mybir.MemoryLocationSet):
            continue
        name = alloc.memorylocations[0].name
        if alloc.kind == "ExternalInput":
            if name != partition_name:
                in_names.append(name)
        elif alloc.kind == "ExternalOutput":
            out_names.append(name)
            out_avals.append(
                jax.core.ShapedArray(
                    tuple(alloc.tensor_shape), mybir.dt.np(alloc.dtype)
                )
            )
    n_params = len(in_names)
    in_names_all = list(in_names) + out_names
    if partition_name is not None:
        in_names_all.append(partition_name)

    def _body(*args):
        operands = list(args)
        if partition_name is not None:
            operands.append(bass2jax.partition_id_tensor())
        return tuple(
            bass2jax._bass_exec_p.bind(
                *operands,
                out_avals=tuple(out_avals),
                in_names=tuple(in_names_all),
                out_names=tuple(out_names),
                lowering_input_output_aliases=(),
                sim_require_finite=True,
                sim_require_nnan=True,
                nc=nc,
            )
        )

    devices = jax.devices()[:NCORES]
    mesh = Mesh(np.asarray(devices), ("core",))
    nouts = len(out_names)
    sharded = jax.jit(
        shard_map(
            _body,
            mesh=mesh,
            in_specs=(PartitionSpec("core"),) * (n_params + nouts),
            out_specs=(PartitionSpec("core"),) * nouts,
            check_rep=False,
        ),
        keep_unused=True,
    )

    def run(in_maps):
        concat_in = [
            np.concatenate([np.asarray(in_maps[c][nm]) for c in range(NCORES)], axis=0)
            for nm in in_names
        ]
        concat_zero = [
            np.zeros((NCORES * a.shape[0], *a.shape[1:]), a.dtype) for a in out_avals
        ]
        outs = sharded(*concat_in, *concat_zero)
        return [
            {
                nm: np.asarray(outs[i]).reshape(NCORES, *out_avals[i].shape)[c]
                for i, nm in enumerate(out_names)
            }
            for c in range(NCORES)
        ]

    run.sharded = sharded
    run.in_names = list(in_names)
    run.out_shapes = [(tuple(a.shape), a.dtype) for a in out_avals]
    _RUNNERS[nc] = run
    return run


def kernel(x, kernel, recurrent_kernel):
    in_maps, kcorr = prepare_inputs(x, kernel, recurrent_kernel)
    nc = _get_nc(kcorr)
    results = _get_runner(nc)(in_maps)
    return assemble_output(results)

